# revision 17
# baseline (speedup 1.0000x reference)
"""BertSelfAttention with relative-position key/value biases on 8 TRN2 NeuronCores.

Sharding: core c -> batch c//2, heads (c%2)*8 .. +8  (8 independent (b,h) pairs/core).
Per head the kernel computes scoresT[j,i] = k_j . q_i (+ banded rel-pos key bias,
inserted via GPSIMD local_scatter shear + bf16 transpose-matmuls accumulating into
PSUM), exp via ScalarE (mask bias folded into the activation bias operand), then
ctxT[d,i] = sum_j v'[j,d] probsT[j,i] where v' carries a ones-column so row 64 of
the PSUM accumulator is the softmax normalizer. The banded value term is recomputed
in [i,*] coords (narrow matmuls + exp + un-shear local_scatter) and accumulated into
the same PSUM via Wrv^T matmuls. Normalization happens on-device; the host only
shards inputs / gathers output shards (plus the constant bv offset).
"""

import os
import sys

sys.path.insert(0, "/opt/trn_rl_repo")

import numpy as np

import concourse.bass as bass
import concourse.bacc as bacc
import concourse.mybir as mybir
from concourse import bass_utils
from concourse.tile import TileContext
from concourse import library_config

F32 = mybir.dt.float32
F32R = mybir.dt.float32r
BF16 = mybir.dt.bfloat16
I16 = mybir.dt.int16
AF = mybir.ActivationFunctionType

B, S, HID, H, DH = 4, 1024, 1024, 16, 64
WK = WV = 64
NW = 2 * WK + 1  # 129
NCORES = 8
NHC = 8          # heads per core
ST = S // 128    # 8 seq tiles
KT = HID // 128  # 8 contraction tiles
NEG = -1.0e30

LAST_EXEC_NS = None
LAST_RESULTS = None


def _build_nc():
    nc = bacc.Bacc()

    # ---- external I/O (per-core shards) ----
    xt_e = nc.declare_dram_parameter("xt", [HID, S], F32, isOutput=False)
    wqt_e = nc.declare_dram_parameter("wqt", [HID, 512], F32, isOutput=False)
    wkt_e = nc.declare_dram_parameter("wkt", [HID, 512], F32, isOutput=False)
    wvt_e = nc.declare_dram_parameter("wvt", [HID, 512], F32, isOutput=False)
    bqc_e = nc.declare_dram_parameter("bq_cols", [128, 4], F32, isOutput=False)
    bkc_e = nc.declare_dram_parameter("bk_cols", [128, 4], F32, isOutput=False)
    wrkt_e = nc.declare_dram_parameter("wrkt", [128, 256], F32, isOutput=False)
    wrvta_e = nc.declare_dram_parameter("wrvt_a", [128, 65], F32, isOutput=False)
    wrvtb_e = nc.declare_dram_parameter("wrvt_b", [4, 65], F32, isOutput=False)
    mbp_e = nc.declare_dram_parameter("mbias_pad", [1, S + 128], F32, isOutput=False)
    mbc_e = nc.declare_dram_parameter("mbias_cols", [128, ST], F32, isOutput=False)
    iden_e = nc.declare_dram_parameter("identity", [128, 128], F32, isOutput=False)
    insx_e = nc.declare_dram_parameter("ins_idx", [128, 132], I16, isOutput=False)
    valx_e = nc.declare_dram_parameter("val_idx", [128, 256], I16, isOutput=False)
    ones_e = nc.declare_dram_parameter("ones_row", [1, 128], F32, isOutput=False)
    out_e = nc.declare_dram_parameter("out", [NHC, DH, S], F32, isOutput=True)

    with TileContext(nc) as tc, nc.allow_low_precision(
        reason="float32r rounding copies feeding the PE; bf16 probs/corrections"
    ):
        with (
            tc.tile_pool(name="const", bufs=1) as cpool,
            tc.tile_pool(name="persist", bufs=1) as ppool,
        ):
            # ---- constants into SBUF ----
            wrkt_f = cpool.tile([128, 256], F32, tag="wrkt_f")
            nc.sync.dma_start(out=wrkt_f[:], in_=wrkt_e[:])
            wrkt = cpool.tile([128, 256], F32R, tag="wrkt")
            nc.vector.tensor_copy(wrkt[:], wrkt_f[:])
            wrvta_f = cpool.tile([128, 65], F32, tag="wrvta_f")
            nc.sync.dma_start(out=wrvta_f[:], in_=wrvta_e[:])
            wrvtb_f = cpool.tile([4, 65], F32, tag="wrvtb_f")
            nc.sync.dma_start(out=wrvtb_f[:], in_=wrvtb_e[:])
            mbp_f = cpool.tile([1, S + 128], F32, tag="mbp_f")
            nc.sync.dma_start(out=mbp_f[:], in_=mbp_e[:])
            mbp = cpool.tile([1, S + 128], F32R, tag="mbp")
            nc.vector.tensor_copy(mbp[:], mbp_f[:])
            mbc = cpool.tile([128, ST], F32, tag="mbc")
            nc.sync.dma_start(out=mbc[:], in_=mbc_e[:])
            iden_f = cpool.tile([128, 128], F32, tag="iden_f")
            nc.sync.dma_start(out=iden_f[:], in_=iden_e[:])
            insx = cpool.tile([128, 132], I16, tag="insx")
            nc.sync.dma_start(out=insx[:], in_=insx_e[:])
            valx = cpool.tile([128, 256], I16, tag="valx")
            nc.sync.dma_start(out=valx[:], in_=valx_e[:])
            ones_f = cpool.tile([1, 128], F32, tag="ones_f")
            nc.sync.dma_start(out=ones_f[:], in_=ones_e[:])
            ones = cpool.tile([1, 128], F32R, tag="ones")
            nc.vector.tensor_copy(ones[:], ones_f[:])
            bqc = cpool.tile([128, 4], F32, tag="bqc")
            nc.sync.dma_start(out=bqc[:], in_=bqc_e[:])
            bkc = cpool.tile([128, 4], F32, tag="bkc")
            nc.sync.dma_start(out=bkc[:], in_=bkc_e[:])

            iden = cpool.tile([128, 128], BF16, tag="iden")
            nc.vector.tensor_copy(iden[:], iden_f[:])
            wrvt_a = cpool.tile([128, 65], BF16, tag="wrvt_a")
            nc.vector.tensor_copy(wrvt_a[:], wrvta_f[:])
            wrvt_b = cpool.tile([4, 65], BF16, tag="wrvt_b")
            nc.vector.tensor_copy(wrvt_b[:], wrvtb_f[:])

            # ---- persistent activations ----
            qt = [ppool.tile([128, S], F32R, tag=f"qt{t}", name=f"qt{t}") for t in range(4)]
            kt = [ppool.tile([128, S + 128], F32R, tag=f"kt{t}", name=f"kt{t}") for t in range(4)]
            vsb = [ppool.tile([128, 8 * 65], BF16, tag=f"v{j}", name=f"v{j}") for j in range(ST)]

            # zero k padding columns (64 each side)
            for t in range(4):
                nc.vector.memset(kt[t][:, 0:64].bitcast(F32), 0.0)
                nc.vector.memset(kt[t][:, S + 64 : S + 128].bitcast(F32), 0.0)

            # ---- projections ----
            with (
                tc.tile_pool(name="xw", bufs=1) as xw,
                tc.tile_pool(name="proj_ps", bufs=2, space="PSUM") as pps,
            ):
                xts = [xw.tile([128, S], F32R, tag=f"x{k}", name=f"x{k}") for k in range(KT)]
                wq = [xw.tile([128, 512], F32R, tag=f"wq{k}", name=f"wq{k}") for k in range(KT)]
                wk_ = [xw.tile([128, 512], F32R, tag=f"wk{k}", name=f"wk{k}") for k in range(KT)]
                wv = [xw.tile([128, 512], F32R, tag=f"wv{k}", name=f"wv{k}") for k in range(KT)]
                for k in range(KT):
                    xs = xw.tile([128, S], F32, tag="xstage", bufs=2, name=f"xs{k}")
                    nc.sync.dma_start(out=xs[:], in_=xt_e[k * 128 : (k + 1) * 128, :])
                    nc.vector.tensor_copy(xts[k][:], xs[:])
                for k in range(KT):
                    for wdst, wsrc in ((wq[k], wqt_e), (wk_[k], wkt_e), (wv[k], wvt_e)):
                        ws = xw.tile([128, 512], F32, tag="wstage", bufs=3, name=f"ws{k}")
                        nc.sync.dma_start(out=ws[:], in_=wsrc[k * 128 : (k + 1) * 128, :])
                        nc.vector.tensor_copy(wdst[:], ws[:])

                # qT / kT feature-major [512, S]
                for t in range(4):
                    for nch in range(2):
                        ps_q = pps.tile([128, 512], F32, tag="pp")
                        for k in range(KT):
                            nc.tensor.matmul(
                                ps_q[:],
                                wq[k][:, t * 128 : (t + 1) * 128],
                                xts[k][:, nch * 512 : (nch + 1) * 512],
                                start=(k == 0), stop=(k == KT - 1),
                            )
                        nc.scalar.activation(
                            qt[t][:, nch * 512 : (nch + 1) * 512],
                            ps_q[:],
                            AF.Identity,
                            bias=bqc[:, t : t + 1],
                        )
                        ps_k = pps.tile([128, 512], F32, tag="pp")
                        for k in range(KT):
                            nc.tensor.matmul(
                                ps_k[:],
                                wk_[k][:, t * 128 : (t + 1) * 128],
                                xts[k][:, nch * 512 : (nch + 1) * 512],
                                start=(k == 0), stop=(k == KT - 1),
                            )
                        nc.scalar.activation(
                            kt[t][:, 64 + nch * 512 : 64 + (nch + 1) * 512],
                            ps_k[:],
                            AF.Identity,
                            bias=bkc[:, t : t + 1],
                        )

                # v seq-major, strided per-head layout (65 cols/head, col 64 = ones)
                for j in range(ST):
                    ps_v = pps.tile([128, 512], F32, tag="pp")
                    for k in range(KT):
                        nc.tensor.matmul(
                            ps_v[:],
                            xts[k][:, j * 128 : (j + 1) * 128],
                            wv[k][:],
                            start=(k == 0), stop=(k == KT - 1),
                        )
                    dst = vsb[j][:].rearrange("p (h d) -> p h d", h=8, d=65)
                    nc.vector.tensor_copy(
                        dst[:, :, 0:64],
                        ps_v[:].rearrange("p (h d) -> p h d", h=8, d=64),
                    )
                    nc.vector.memset(dst[:, :, 64:65], 1.0)

            # ---- per (head) attention ----
            with (
                tc.tile_pool(name="wt", bufs=2) as wtp,
                tc.tile_pool(name="probs", bufs=2) as prp,
                tc.tile_pool(name="sm", bufs=2) as smp,
                tc.tile_pool(name="st_ps", bufs=2, space="PSUM") as stps,
                tc.tile_pool(name="ctx_ps", bufs=1, space="PSUM") as ctxps,
                tc.tile_pool(name="small_ps", bufs=2, space="PSUM") as smps,
            ):
                for hh in range(NHC):
                    tq = qt[hh // 2]
                    tk = kt[hh // 2]
                    base = (hh % 2) * 64

                    # A: a_k then shear-scatter into window coords
                    wts = []
                    for it in range(ST):
                        akps = smps.tile([128, 256], F32, tag="sm")
                        nc.tensor.matmul(
                            akps[:],
                            tq[base : base + 64, it * 128 : (it + 1) * 128],
                            wrkt[base : base + 64, :],
                            start=True, stop=True,
                        )
                        akb = smp.tile([128, 132], BF16, tag="akb")
                        nc.vector.tensor_copy(akb[:], akps[:, 0:132])
                        w_t = wtp.tile([128, 384], BF16, tag=f"w{it}", name=f"w{it}")
                        nc.gpsimd.local_scatter(
                            w_t[:], akb[:], insx[:],
                            channels=128, num_elems=384, num_idxs=132,
                        )
                        wts.append(w_t)

                    # B: scoresT per j-tile -> probsT
                    # Each write: (part_lo, part_hi, col_lo, col_hi, lhsT, rhs, tile_pos)
                    prbs = []
                    for jt in range(ST):
                        st = stps.tile([128, S], F32, tag="st")
                        writes = []
                        for ich in range(2):
                            writes.append((
                                0, 128, ich * 512, (ich + 1) * 512,
                                tk[base : base + 64, 64 + jt * 128 : 64 + (jt + 1) * 128],
                                tq[base : base + 64, ich * 512 : (ich + 1) * 512],
                                None, True,
                            ))
                        # banded rel-pos key bias: transposed window pieces accumulate.
                        # W_t col c' = j - t*128 + 128 in the 384-wide padded window;
                        # out-of-window columns are scatter-zeroed so every piece is a
                        # full 128-partition matmul adding zeros where invalid.
                        writes.append((
                            0, 128, jt * 128, (jt + 1) * 128,
                            wts[jt][:, 128:256], iden[:], None, False,
                        ))
                        if jt > 0:
                            writes.append((
                                0, 128, (jt - 1) * 128, jt * 128,
                                wts[jt - 1][:, 256:384], iden[:], None, False,
                            ))
                        if jt < ST - 1:
                            writes.append((
                                0, 128, (jt + 1) * 128, (jt + 2) * 128,
                                wts[jt + 1][:, 0:128], iden[:], None, False,
                            ))
                        # last writer of each PSUM bank carries stop=True
                        last_in_bank = {}
                        for i, wr in enumerate(writes):
                            last_in_bank[wr[2] // 512] = i
                        for i, (plo, phi, clo, chi, lh, rh, tpos, sflag) in enumerate(writes):
                            nc.tensor.matmul(
                                st[plo:phi, clo:chi], lh, rh,
                                start=sflag, stop=(i in last_in_bank.values()),
                                tile_position=tpos,
                            )
                        pr = prp.tile([128, S], BF16, tag=f"pr{jt}", name=f"pr{jt}")
                        nc.scalar.activation(
                            pr[:], st[:], AF.Exp, bias=mbc[:, jt : jt + 1]
                        )
                        prbs.append(pr)

                    # C: PV (+ softmax sums in row 64)
                    ctx = ctxps.tile([65, S], F32, tag="ctx")
                    for jt in range(ST):
                        for ich in range(2):
                            nc.tensor.matmul(
                                ctx[:, ich * 512 : (ich + 1) * 512],
                                vsb[jt][:, hh * 65 : (hh + 1) * 65],
                                prbs[jt][:, ich * 512 : (ich + 1) * 512],
                                start=(jt == 0), stop=False,
                            )

                    # D: banded value term, recomputed in [i,*] coords
                    for it in range(ST):
                        bps = smps.tile([128, 256], F32, tag="sm")
                        nc.tensor.matmul(
                            bps[:],
                            tq[base : base + 64, it * 128 : (it + 1) * 128],
                            tk[base : base + 64, it * 128 : it * 128 + 256],
                            start=True, stop=False,
                        )
                        nc.tensor.matmul(
                            bps[:],
                            ones[0:1, :],
                            mbp[0:1, it * 128 : it * 128 + 256],
                            start=False, stop=True,
                        )
                        nc.vector.tensor_add(bps[:], bps[:], wts[it][:, 64:320])
                        ppu = smp.tile([128, 256], BF16, tag="ppu")
                        nc.scalar.activation(ppu[:], bps[:], AF.Exp)
                        if it == 0:
                            nc.vector.memset(ppu[:, 0:64], 0.0)
                        if it == ST - 1:
                            nc.vector.memset(ppu[:, 192:256], 0.0)
                        pp = smp.tile([128, 256], BF16, tag="pp")
                        nc.gpsimd.local_scatter(
                            pp[:], ppu[:], valx[:],
                            channels=128, num_elems=256, num_idxs=256,
                        )
                        ppt = smps.tile([128, 256], F32, tag="sm")
                        nc.tensor.matmul(ppt[0:128, 0:128], pp[:, 0:128], iden[:],
                                         start=True, stop=True)
                        nc.tensor.matmul(ppt[0:4, 128:256], pp[:, 128:132], iden[:],
                                         start=True, stop=True)
                        pa = smp.tile([128, 128], BF16, tag="pa")
                        nc.vector.tensor_copy(pa[:], ppt[0:128, 0:128])
                        pb = smp.tile([4, 128], BF16, tag="pb")
                        nc.vector.tensor_copy(pb[:], ppt[0:4, 128:256])
                        nc.tensor.matmul(
                            ctx[0:65, it * 128 : (it + 1) * 128],
                            wrvt_a[:], pa[:], start=False, stop=False,
                        )
                        nc.tensor.matmul(
                            ctx[0:65, it * 128 : (it + 1) * 128],
                            wrvt_b[:], pb[:], start=False,
                            stop=(it == 3 or it == ST - 1),
                        )

                    # E: normalize + write out
                    r = smp.tile([1, S], F32R, tag="r")
                    nc.vector.reciprocal(r[:], ctx[64:65, :])
                    rb = stps.tile([64, S], F32, tag="st")
                    for ich in range(2):
                        nc.tensor.matmul(
                            rb[:, ich * 512 : (ich + 1) * 512],
                            ones[0:1, 0:64],
                            r[0:1, ich * 512 : (ich + 1) * 512],
                            start=True, stop=True,
                        )
                    rbs = smp.tile([64, S], F32, tag="rbs")
                    nc.vector.tensor_copy(rbs[:], rb[:])
                    o = smp.tile([64, S], F32, tag="o")
                    nc.vector.tensor_mul(o[:], ctx[0:64, :], rbs[:])
                    nc.sync.dma_start(out=out_e[hh], in_=o[:])

    nc.compile()
    return nc


_NC_CACHE = None


def _get_nc():
    global _NC_CACHE
    if _NC_CACHE is None:
        _NC_CACHE = _build_nc()
    return _NC_CACHE


def _host_prep(inputs):
    hs = np.asarray(inputs["hidden_states"], np.float32)
    am = np.asarray(inputs["attention_mask"], np.float32)
    Wq = np.asarray(inputs["Wq"], np.float32)
    bq = np.asarray(inputs["bq"], np.float32)
    Wk = np.asarray(inputs["Wk"], np.float32)
    bk = np.asarray(inputs["bk"], np.float32)
    Wv = np.asarray(inputs["Wv"], np.float32)
    Wrk = np.asarray(inputs["Wrk"], np.float32)
    Wrv = np.asarray(inputs["Wrv"], np.float32)

    wrkt = np.zeros((128, 256), np.float32)
    wrkt[0:64, 0:NW] = Wrk.T
    wrkt[64:128, 0:NW] = Wrk.T
    wrvt = np.zeros((132, 65), np.float32)
    wrvt[0:NW, 0:64] = Wrv.T
    wrvt_a = np.ascontiguousarray(wrvt[0:128])
    wrvt_b = np.ascontiguousarray(wrvt[128:132])

    iden = np.eye(128, dtype=np.float32)
    p = np.arange(128)[:, None]
    w = np.arange(132)[None, :]
    ins_idx = np.where(w <= 128, p + w + 64, -1).astype(np.int16)
    c = np.arange(256)[None, :]
    d = c - p
    val_idx = np.where((d >= 0) & (d <= 128), d, -1).astype(np.int16)
    ones_row = np.ones((1, 128), np.float32)

    in_maps = []
    for core in range(NCORES):
        b = core // 2
        h0 = (core % 2) * NHC
        fsl = slice(h0 * DH, h0 * DH + 512)
        mb = (1.0 - am[b]) * NEG
        mbp = np.zeros((1, S + 128), np.float32)
        mbp[0, 64 : 64 + S] = mb
        in_maps.append({
            "xt": np.ascontiguousarray(hs[b].T),
            "wqt": np.ascontiguousarray(Wq[fsl].T),
            "wkt": np.ascontiguousarray(Wk[fsl].T),
            "wvt": np.ascontiguousarray(Wv[fsl].T),
            "bq_cols": np.ascontiguousarray(bq[fsl].reshape(4, 128).T),
            "bk_cols": np.ascontiguousarray(bk[fsl].reshape(4, 128).T),
            "wrkt": wrkt,
            "wrvt_a": wrvt_a,
            "wrvt_b": wrvt_b,
            "mbias_pad": mbp,
            "mbias_cols": np.ascontiguousarray(mb.reshape(ST, 128).T),
            "identity": iden,
            "ins_idx": ins_idx,
            "val_idx": val_idx,
            "ones_row": ones_row,
        })
    return in_maps


def _assemble(results, inputs):
    bv = np.asarray(inputs["bv"], np.float32)
    full = np.empty((B, S, H * DH), np.float32)
    for core in range(NCORES):
        b = core // 2
        h0 = (core % 2) * NHC
        o = results[core]["out"]  # [NHC, DH, S]
        for hh in range(NHC):
            h = h0 + hh
            full[b, :, h * DH : (h + 1) * DH] = o[hh].T
    full += bv[None, None, :]
    return full


def kernel(**inputs):
    global LAST_EXEC_NS, LAST_RESULTS
    nc = _get_nc()
    in_maps = _host_prep(inputs)
    trace = bool(int(os.environ.get("KERNEL_TRACE", "0")))
    res = bass_utils.run_bass_kernel_spmd(
        nc, in_maps, core_ids=list(range(NCORES)), trace=trace
    )
    LAST_EXEC_NS = res.exec_time_ns
    LAST_RESULTS = res
    return _assemble(res.results, inputs)


# revision 20
# speedup vs baseline: 1.1188x; 1.1188x over previous
"""BertSelfAttention with relative-position key/value biases on 8 TRN2 NeuronCores.

Sharding: core c -> batch c//2, heads (c%2)*8 .. +8  (8 independent (b,h) pairs/core).
Per head the kernel computes scoresT[j,i] = k_j . q_i (+ banded rel-pos key bias,
inserted via GPSIMD local_scatter shear + bf16 transpose-matmuls accumulating into
PSUM), exp via ScalarE (mask bias folded into the activation bias operand), then
ctxT[d,i] = sum_j v'[j,d] probsT[j,i] where v' carries a ones-column so row 64 of
the PSUM accumulator is the softmax normalizer. The banded value term is recomputed
in [i,*] coords (narrow matmuls + exp + un-shear local_scatter) and accumulated into
the same PSUM via Wrv^T matmuls. Normalization happens on-device; the host only
shards inputs / gathers output shards (plus the constant bv offset).
"""

import os
import sys

sys.path.insert(0, "/opt/trn_rl_repo")

import numpy as np

import concourse.bass as bass
import concourse.bacc as bacc
import concourse.mybir as mybir
from concourse import bass_utils
from concourse.tile import TileContext
from concourse import library_config

F32 = mybir.dt.float32
F32R = mybir.dt.float32r
BF16 = mybir.dt.bfloat16
I16 = mybir.dt.int16
AF = mybir.ActivationFunctionType

B, S, HID, H, DH = 4, 1024, 1024, 16, 64
WK = WV = 64
NW = 2 * WK + 1  # 129
NCORES = 8
NHC = 8          # heads per core
ST = S // 128    # 8 seq tiles
KT = HID // 128  # 8 contraction tiles
NEG = -1.0e30

LAST_EXEC_NS = None
LAST_RESULTS = None


def _build_nc(with_mask_bias=False):
    nc = bacc.Bacc()

    # ---- external I/O (per-core shards) ----
    xt_e = nc.declare_dram_parameter("xt", [HID, S], F32, isOutput=False)
    wqt_e = nc.declare_dram_parameter("wqt", [HID, 512], F32, isOutput=False)
    wkt_e = nc.declare_dram_parameter("wkt", [HID, 512], F32, isOutput=False)
    wvt_e = nc.declare_dram_parameter("wvt", [HID, 512], F32, isOutput=False)
    bqc_e = nc.declare_dram_parameter("bq_cols", [128, 4], F32, isOutput=False)
    bkc_e = nc.declare_dram_parameter("bk_cols", [128, 4], F32, isOutput=False)
    wrkt_e = nc.declare_dram_parameter("wrkt", [128, 256], F32, isOutput=False)
    wrvta_e = nc.declare_dram_parameter("wrvt_a", [128, 65], F32, isOutput=False)
    wrvtb_e = nc.declare_dram_parameter("wrvt_b", [4, 65], F32, isOutput=False)
    mbp_e = nc.declare_dram_parameter("mbias_pad", [1, S + 128], F32, isOutput=False)
    mbc_e = nc.declare_dram_parameter("mbias_cols", [128, ST], F32, isOutput=False)
    iden_e = nc.declare_dram_parameter("identity", [128, 128], F32, isOutput=False)
    insx_e = nc.declare_dram_parameter("ins_idx", [128, 132], I16, isOutput=False)
    valx_e = nc.declare_dram_parameter("val_idx", [128, 256], I16, isOutput=False)
    ones_e = nc.declare_dram_parameter("ones_row", [1, 128], F32, isOutput=False)
    out_e = nc.declare_dram_parameter("out", [NHC, DH, S], F32, isOutput=True)

    with TileContext(nc) as tc, nc.allow_low_precision(
        reason="float32r rounding copies feeding the PE; bf16 probs/corrections"
    ):
        with (
            tc.tile_pool(name="const", bufs=1) as cpool,
            tc.tile_pool(name="persist", bufs=1) as ppool,
        ):
            # ---- constants into SBUF ----
            wrkt_f = cpool.tile([128, 256], F32, tag="wrkt_f")
            nc.sync.dma_start(out=wrkt_f[:], in_=wrkt_e[:])
            wrkt = cpool.tile([128, 256], F32R, tag="wrkt")
            nc.vector.tensor_copy(wrkt[:], wrkt_f[:])
            wrvta_f = cpool.tile([128, 65], F32, tag="wrvta_f")
            nc.sync.dma_start(out=wrvta_f[:], in_=wrvta_e[:])
            wrvtb_f = cpool.tile([4, 65], F32, tag="wrvtb_f")
            nc.sync.dma_start(out=wrvtb_f[:], in_=wrvtb_e[:])
            mbp_f = cpool.tile([1, S + 128], F32, tag="mbp_f")
            nc.sync.dma_start(out=mbp_f[:], in_=mbp_e[:])
            mbp = cpool.tile([1, S + 128], F32R, tag="mbp")
            nc.vector.tensor_copy(mbp[:], mbp_f[:])
            mbc = cpool.tile([128, ST], F32, tag="mbc")
            nc.sync.dma_start(out=mbc[:], in_=mbc_e[:])
            iden_f = cpool.tile([128, 128], F32, tag="iden_f")
            nc.sync.dma_start(out=iden_f[:], in_=iden_e[:])
            insx = cpool.tile([128, 132], I16, tag="insx")
            nc.sync.dma_start(out=insx[:], in_=insx_e[:])
            valx = cpool.tile([128, 256], I16, tag="valx")
            nc.sync.dma_start(out=valx[:], in_=valx_e[:])
            ones_f = cpool.tile([1, 128], F32, tag="ones_f")
            nc.sync.dma_start(out=ones_f[:], in_=ones_e[:])
            ones = cpool.tile([1, 128], F32R, tag="ones")
            nc.vector.tensor_copy(ones[:], ones_f[:])
            bqc = cpool.tile([128, 4], F32, tag="bqc")
            nc.sync.dma_start(out=bqc[:], in_=bqc_e[:])
            bkc = cpool.tile([128, 4], F32, tag="bkc")
            nc.sync.dma_start(out=bkc[:], in_=bkc_e[:])

            iden = cpool.tile([128, 128], BF16, tag="iden")
            nc.vector.tensor_copy(iden[:], iden_f[:])
            wrvt_a = cpool.tile([128, 65], BF16, tag="wrvt_a")
            nc.vector.tensor_copy(wrvt_a[:], wrvta_f[:])
            wrvt_b = cpool.tile([4, 65], BF16, tag="wrvt_b")
            nc.vector.tensor_copy(wrvt_b[:], wrvtb_f[:])

            # ---- persistent activations ----
            qt = [ppool.tile([128, S], F32R, tag=f"qt{t}", name=f"qt{t}") for t in range(4)]
            kt = [ppool.tile([128, S + 128], F32R, tag=f"kt{t}", name=f"kt{t}") for t in range(4)]
            vsb = [ppool.tile([128, 8 * 65], BF16, tag=f"v{j}", name=f"v{j}") for j in range(ST)]

            # zero k padding columns (64 each side)
            for t in range(4):
                nc.vector.memset(kt[t][:, 0:64].bitcast(F32), 0.0)
                nc.vector.memset(kt[t][:, S + 64 : S + 128].bitcast(F32), 0.0)

            # ---- projections ----
            with (
                tc.tile_pool(name="xw", bufs=1) as xw,
                tc.tile_pool(name="proj_ps", bufs=1, space="PSUM") as pps,
            ):
                xts = [xw.tile([128, S], F32R, tag=f"x{k}", name=f"x{k}") for k in range(KT)]
                wq = [xw.tile([128, 512], F32R, tag=f"wq{k}", name=f"wq{k}") for k in range(KT)]
                wk_ = [xw.tile([128, 512], F32R, tag=f"wk{k}", name=f"wk{k}") for k in range(KT)]
                wv = [xw.tile([128, 512], F32R, tag=f"wv{k}", name=f"wv{k}") for k in range(KT)]
                for k in range(KT):
                    xs = xw.tile([128, S], F32, tag="xstage", bufs=2, name=f"xs{k}")
                    nc.sync.dma_start(out=xs[:], in_=xt_e[k * 128 : (k + 1) * 128, :])
                    nc.vector.tensor_copy(xts[k][:], xs[:])
                for k in range(KT):
                    for wdst, wsrc in ((wq[k], wqt_e), (wk_[k], wkt_e), (wv[k], wvt_e)):
                        ws = xw.tile([128, 512], F32, tag="wstage", bufs=3, name=f"ws{k}")
                        nc.sync.dma_start(out=ws[:], in_=wsrc[k * 128 : (k + 1) * 128, :])
                        nc.vector.tensor_copy(wdst[:], ws[:])

                # qT / kT feature-major [512, S]; k-outer so the PE starts as
                # soon as the first K-chunk lands, accumulating into 8 banks.
                for wsrcs, dsts, biases, seq_major in (
                    (wq, qt, bqc, False),
                    (wk_, kt, bkc, False),
                    (wv, vsb, None, True),
                ):
                    ps8 = [
                        pps.tile([128, 512], F32, tag=f"pj{i}", name=f"pj{i}")
                        for i in range(8)
                    ]
                    for k in range(KT):
                        for i in range(8):
                            if seq_major:
                                lh = xts[k][:, i * 128 : (i + 1) * 128]
                                rh = wsrcs[k][:]
                            else:
                                t, nch = i // 2, i % 2
                                lh = wsrcs[k][:, t * 128 : (t + 1) * 128]
                                rh = xts[k][:, nch * 512 : (nch + 1) * 512]
                            nc.tensor.matmul(
                                ps8[i][:], lh, rh,
                                start=(k == 0), stop=(k == KT - 1),
                            )
                    for i in range(8):
                        if seq_major:
                            dst = dsts[i][:].rearrange("p (h d) -> p h d", h=8, d=65)
                            nc.vector.tensor_copy(
                                dst[:, :, 0:64],
                                ps8[i][:].rearrange("p (h d) -> p h d", h=8, d=64),
                            )
                            nc.vector.memset(dst[:, :, 64:65], 1.0)
                        else:
                            t, nch = i // 2, i % 2
                            off = 0 if dsts is qt else 64
                            nc.scalar.activation(
                                dsts[t][:, off + nch * 512 : off + (nch + 1) * 512],
                                ps8[i][:],
                                AF.Identity,
                                bias=biases[:, t : t + 1],
                            )

            # ---- per (head) attention ----
            with (
                tc.tile_pool(name="wt", bufs=3) as wtp,
                tc.tile_pool(name="probs", bufs=2) as prp,
                tc.tile_pool(name="sm", bufs=2) as smp,
                tc.tile_pool(name="st_ps", bufs=2, space="PSUM") as stps,
                tc.tile_pool(name="ctx_ps", bufs=1, space="PSUM") as ctxps,
                tc.tile_pool(name="small_ps", bufs=2, space="PSUM") as smps,
            ):
                for hh in range(NHC):
                    tq = qt[hh // 2]
                    tk = kt[hh // 2]
                    base = (hh % 2) * 64

                    # A: a_k then shear-scatter into window coords
                    wts = []
                    for it in range(ST):
                        akps = smps.tile([128, 256], F32, tag="sm")
                        nc.tensor.matmul(
                            akps[:],
                            tq[base : base + 64, it * 128 : (it + 1) * 128],
                            wrkt[base : base + 64, :],
                            start=True, stop=True,
                        )
                        akb = smp.tile([128, 132], BF16, tag="akb")
                        nc.vector.tensor_copy(akb[:], akps[:, 0:132])
                        w_t = wtp.tile([128, 384], BF16, tag=f"w{it}", name=f"w{it}")
                        nc.gpsimd.local_scatter(
                            w_t[:], akb[:], insx[:],
                            channels=128, num_elems=384, num_idxs=132,
                        )
                        wts.append(w_t)

                    # B: scoresT per j-tile -> probsT
                    # Each write: (part_lo, part_hi, col_lo, col_hi, lhsT, rhs, tile_pos)
                    prbs = []
                    for jt in range(ST):
                        st = stps.tile([128, S], F32, tag="st")
                        writes = []
                        for ich in range(2):
                            writes.append((
                                0, 128, ich * 512, (ich + 1) * 512,
                                tk[base : base + 64, 64 + jt * 128 : 64 + (jt + 1) * 128],
                                tq[base : base + 64, ich * 512 : (ich + 1) * 512],
                                None, True,
                            ))
                        # banded rel-pos key bias: transposed window pieces accumulate.
                        # W_t col c' = j - t*128 + 128 in the 384-wide padded window;
                        # out-of-window columns are scatter-zeroed so every piece is a
                        # full 128-partition matmul adding zeros where invalid.
                        writes.append((
                            0, 128, jt * 128, (jt + 1) * 128,
                            wts[jt][:, 128:256], iden[:], None, False,
                        ))
                        if jt > 0:
                            writes.append((
                                0, 128, (jt - 1) * 128, jt * 128,
                                wts[jt - 1][:, 256:384], iden[:], None, False,
                            ))
                        if jt < ST - 1:
                            writes.append((
                                0, 128, (jt + 1) * 128, (jt + 2) * 128,
                                wts[jt + 1][:, 0:128], iden[:], None, False,
                            ))
                        # last writer of each PSUM bank carries stop=True
                        last_in_bank = {}
                        for i, wr in enumerate(writes):
                            last_in_bank[wr[2] // 512] = i
                        for i, (plo, phi, clo, chi, lh, rh, tpos, sflag) in enumerate(writes):
                            nc.tensor.matmul(
                                st[plo:phi, clo:chi], lh, rh,
                                start=sflag, stop=(i in last_in_bank.values()),
                                tile_position=tpos,
                            )
                        pr = prp.tile([128, S], BF16, tag=f"pr{jt}", name=f"pr{jt}")
                        nc.scalar.activation(
                            pr[:], st[:], AF.Exp, bias=mbc[:, jt : jt + 1]
                        )
                        prbs.append(pr)

                    # C: PV (+ softmax sums in row 64)
                    ctx = ctxps.tile([65, S], F32, tag="ctx")
                    for jt in range(ST):
                        for ich in range(2):
                            nc.tensor.matmul(
                                ctx[:, ich * 512 : (ich + 1) * 512],
                                vsb[jt][:, hh * 65 : (hh + 1) * 65],
                                prbs[jt][:, ich * 512 : (ich + 1) * 512],
                                start=(jt == 0), stop=False,
                            )

                    # D: banded value term, recomputed in [i,*] coords
                    for it in range(ST):
                        bps = smps.tile([128, 256], F32, tag="sm")
                        nc.tensor.matmul(
                            bps[:],
                            tq[base : base + 64, it * 128 : (it + 1) * 128],
                            tk[base : base + 64, it * 128 : it * 128 + 256],
                            start=True, stop=not with_mask_bias,
                        )
                        if with_mask_bias:
                            nc.tensor.matmul(
                                bps[:],
                                ones[0:1, :],
                                mbp[0:1, it * 128 : it * 128 + 256],
                                start=False, stop=True,
                            )
                        nc.vector.tensor_add(bps[:], bps[:], wts[it][:, 64:320])
                        ppu = smp.tile([128, 256], BF16, tag="ppu")
                        nc.scalar.activation(ppu[:], bps[:], AF.Exp)
                        if it == 0:
                            nc.vector.memset(ppu[:, 0:64], 0.0)
                        if it == ST - 1:
                            nc.vector.memset(ppu[:, 192:256], 0.0)
                        pp = smp.tile([128, 256], BF16, tag="pp")
                        nc.gpsimd.local_scatter(
                            pp[:], ppu[:], valx[:],
                            channels=128, num_elems=256, num_idxs=256,
                        )
                        pa = smp.tile([128, 128], BF16, tag="pa")
                        nc.sync.dma_start_transpose(pa[:], pp[:, 0:128])
                        ppt = smps.tile([128, 256], F32, tag="sm")
                        nc.tensor.matmul(ppt[0:4, 0:128], pp[:, 128:132], iden[:],
                                         start=True, stop=True)
                        pb = smp.tile([4, 128], BF16, tag="pb")
                        nc.vector.tensor_copy(pb[:], ppt[0:4, 0:128])
                        nc.tensor.matmul(
                            ctx[0:65, it * 128 : (it + 1) * 128],
                            wrvt_a[:], pa[:], start=False, stop=False,
                        )
                        nc.tensor.matmul(
                            ctx[0:65, it * 128 : (it + 1) * 128],
                            wrvt_b[:], pb[:], start=False,
                            stop=(it == 3 or it == ST - 1),
                        )

                    # E: copy ctx out of PSUM (frees it for the next head),
                    # then normalize entirely off the critical path.
                    cs = smp.tile([65, S], F32, tag="cs")
                    nc.vector.tensor_copy(cs[:], ctx[:])
                    r = smp.tile([1, S], F32R, tag="r")
                    nc.vector.reciprocal(r[:], cs[64:65, :])
                    rb = stps.tile([64, S], F32, tag="st")
                    for ich in range(2):
                        nc.tensor.matmul(
                            rb[:, ich * 512 : (ich + 1) * 512],
                            ones[0:1, 0:64],
                            r[0:1, ich * 512 : (ich + 1) * 512],
                            start=True, stop=True,
                        )
                    o = smp.tile([64, S], F32, tag="o")
                    nc.vector.tensor_mul(o[:], cs[0:64, :], rb[:])
                    nc.sync.dma_start(out=out_e[hh], in_=o[:])

    nc.compile()
    return nc


_NC_CACHE = {}


def _get_nc(with_mask_bias=False):
    if with_mask_bias not in _NC_CACHE:
        _NC_CACHE[with_mask_bias] = _build_nc(with_mask_bias)
    return _NC_CACHE[with_mask_bias]


def _host_prep(inputs):
    hs = np.asarray(inputs["hidden_states"], np.float32)
    am = np.asarray(inputs["attention_mask"], np.float32)
    Wq = np.asarray(inputs["Wq"], np.float32)
    bq = np.asarray(inputs["bq"], np.float32)
    Wk = np.asarray(inputs["Wk"], np.float32)
    bk = np.asarray(inputs["bk"], np.float32)
    Wv = np.asarray(inputs["Wv"], np.float32)
    Wrk = np.asarray(inputs["Wrk"], np.float32)
    Wrv = np.asarray(inputs["Wrv"], np.float32)

    wrkt = np.zeros((128, 256), np.float32)
    wrkt[0:64, 0:NW] = Wrk.T
    wrkt[64:128, 0:NW] = Wrk.T
    wrvt = np.zeros((132, 65), np.float32)
    wrvt[0:NW, 0:64] = Wrv.T
    wrvt_a = np.ascontiguousarray(wrvt[0:128])
    wrvt_b = np.ascontiguousarray(wrvt[128:132])

    iden = np.eye(128, dtype=np.float32)
    p = np.arange(128)[:, None]
    w = np.arange(132)[None, :]
    ins_idx = np.where(w <= 128, p + w + 64, -1).astype(np.int16)
    c = np.arange(256)[None, :]
    d = c - p
    val_idx = np.where((d >= 0) & (d <= 128), d, -1).astype(np.int16)
    ones_row = np.ones((1, 128), np.float32)

    in_maps = []
    for core in range(NCORES):
        b = core // 2
        h0 = (core % 2) * NHC
        fsl = slice(h0 * DH, h0 * DH + 512)
        mb = (1.0 - am[b]) * NEG
        mbp = np.zeros((1, S + 128), np.float32)
        mbp[0, 64 : 64 + S] = mb
        in_maps.append({
            "xt": np.ascontiguousarray(hs[b].T),
            "wqt": np.ascontiguousarray(Wq[fsl].T),
            "wkt": np.ascontiguousarray(Wk[fsl].T),
            "wvt": np.ascontiguousarray(Wv[fsl].T),
            "bq_cols": np.ascontiguousarray(bq[fsl].reshape(4, 128).T),
            "bk_cols": np.ascontiguousarray(bk[fsl].reshape(4, 128).T),
            "wrkt": wrkt,
            "wrvt_a": wrvt_a,
            "wrvt_b": wrvt_b,
            "mbias_pad": mbp,
            "mbias_cols": np.ascontiguousarray(mb.reshape(ST, 128).T),
            "identity": iden,
            "ins_idx": ins_idx,
            "val_idx": val_idx,
            "ones_row": ones_row,
        })
    return in_maps


def _assemble(results, inputs):
    bv = np.asarray(inputs["bv"], np.float32)
    full = np.empty((B, S, H * DH), np.float32)
    for core in range(NCORES):
        b = core // 2
        h0 = (core % 2) * NHC
        o = results[core]["out"]  # [NHC, DH, S]
        for hh in range(NHC):
            h = h0 + hh
            full[b, :, h * DH : (h + 1) * DH] = o[hh].T
    full += bv[None, None, :]
    return full


def kernel(**inputs):
    global LAST_EXEC_NS, LAST_RESULTS
    mask_all_ones = bool(np.all(np.asarray(inputs["attention_mask"]) == 1.0))
    nc = _get_nc(with_mask_bias=not mask_all_ones)
    in_maps = _host_prep(inputs)
    trace = bool(int(os.environ.get("KERNEL_TRACE", "0")))
    res = bass_utils.run_bass_kernel_spmd(
        nc, in_maps, core_ids=list(range(NCORES)), trace=trace
    )
    LAST_EXEC_NS = res.exec_time_ns
    LAST_RESULTS = res
    return _assemble(res.results, inputs)


# revision 22
# speedup vs baseline: 1.3470x; 1.2040x over previous
"""BertSelfAttention with relative-position key/value biases on 8 TRN2 NeuronCores.

Sharding: core c -> batch c//2, heads (c%2)*8 .. +8  (8 independent (b,h) pairs/core).
Per head the kernel computes scoresT[j,i] = k_j . q_i (+ banded rel-pos key bias,
inserted via GPSIMD local_scatter shear + bf16 transpose-matmuls accumulating into
PSUM), exp via ScalarE (mask bias folded into the activation bias operand), then
ctxT[d,i] = sum_j v'[j,d] probsT[j,i] where v' carries a ones-column so row 64 of
the PSUM accumulator is the softmax normalizer. The banded value term is recomputed
in [i,*] coords (narrow matmuls + exp + un-shear local_scatter) and accumulated into
the same PSUM via Wrv^T matmuls. Normalization happens on-device; the host only
shards inputs / gathers output shards (plus the constant bv offset).
"""

import os
import sys

sys.path.insert(0, "/opt/trn_rl_repo")

import numpy as np

import concourse.bass as bass
import concourse.bacc as bacc
import concourse.mybir as mybir
from concourse import bass_utils
from concourse.tile import TileContext
from concourse import library_config

F32 = mybir.dt.float32
F32R = mybir.dt.float32r
BF16 = mybir.dt.bfloat16
I16 = mybir.dt.int16
AF = mybir.ActivationFunctionType

B, S, HID, H, DH = 4, 1024, 1024, 16, 64
WK = WV = 64
NW = 2 * WK + 1  # 129
NCORES = 8
NHC = 8          # heads per core
ST = S // 128    # 8 seq tiles
KT = HID // 128  # 8 contraction tiles
NEG = -1.0e30

LAST_EXEC_NS = None
LAST_RESULTS = None


def _build_nc(with_mask_bias=False):
    nc = bacc.Bacc()

    # ---- external I/O (per-core shards) ----
    xt_e = nc.declare_dram_parameter("xt", [HID, S], F32, isOutput=False)
    wqt_e = nc.declare_dram_parameter("wqt", [HID, 512], F32, isOutput=False)
    wkt_e = nc.declare_dram_parameter("wkt", [HID, 512], F32, isOutput=False)
    wvt_e = nc.declare_dram_parameter("wvt", [HID, 512], F32, isOutput=False)
    bqc_e = nc.declare_dram_parameter("bq_cols", [128, 4], F32, isOutput=False)
    bkc_e = nc.declare_dram_parameter("bk_cols", [128, 4], F32, isOutput=False)
    wrkt_e = nc.declare_dram_parameter("wrkt", [128, 256], F32, isOutput=False)
    wrvta_e = nc.declare_dram_parameter("wrvt_a", [128, 65], F32, isOutput=False)
    wrvtb_e = nc.declare_dram_parameter("wrvt_b", [4, 65], F32, isOutput=False)
    mbp_e = nc.declare_dram_parameter("mbias_pad", [1, S + 128], F32, isOutput=False)
    mbc_e = nc.declare_dram_parameter("mbias_cols", [128, ST], F32, isOutput=False)
    iden_e = nc.declare_dram_parameter("identity", [128, 128], F32, isOutput=False)
    insx_e = nc.declare_dram_parameter("ins_idx", [128, 528], I16, isOutput=False)
    valx_e = nc.declare_dram_parameter("val_idx", [128, 512], I16, isOutput=False)
    ones_e = nc.declare_dram_parameter("ones_row", [1, 128], F32, isOutput=False)
    out_e = nc.declare_dram_parameter("out", [NHC, DH, S], F32, isOutput=True)

    with TileContext(nc) as tc, nc.allow_low_precision(
        reason="float32r rounding copies feeding the PE; bf16 probs/corrections"
    ):
        with (
            tc.tile_pool(name="const", bufs=1) as cpool,
            tc.tile_pool(name="persist", bufs=1) as ppool,
        ):
            # ---- constants into SBUF ----
            wrkt_f = cpool.tile([128, 256], F32, tag="wrkt_f")
            nc.sync.dma_start(out=wrkt_f[:], in_=wrkt_e[:])
            wrkt = cpool.tile([128, 256], F32R, tag="wrkt")
            nc.vector.tensor_copy(wrkt[:], wrkt_f[:])
            wrvta_f = cpool.tile([128, 65], F32, tag="wrvta_f")
            nc.sync.dma_start(out=wrvta_f[:], in_=wrvta_e[:])
            wrvtb_f = cpool.tile([4, 65], F32, tag="wrvtb_f")
            nc.sync.dma_start(out=wrvtb_f[:], in_=wrvtb_e[:])
            mbp_f = cpool.tile([1, S + 128], F32, tag="mbp_f")
            nc.sync.dma_start(out=mbp_f[:], in_=mbp_e[:])
            mbp = cpool.tile([1, S + 128], F32R, tag="mbp")
            nc.vector.tensor_copy(mbp[:], mbp_f[:])
            mbc = cpool.tile([128, ST], F32, tag="mbc")
            nc.sync.dma_start(out=mbc[:], in_=mbc_e[:])
            iden_f = cpool.tile([128, 128], F32, tag="iden_f")
            nc.sync.dma_start(out=iden_f[:], in_=iden_e[:])
            insx = cpool.tile([128, 528], I16, tag="insx")
            nc.sync.dma_start(out=insx[:], in_=insx_e[:])
            valx = cpool.tile([128, 512], I16, tag="valx")
            nc.sync.dma_start(out=valx[:], in_=valx_e[:])
            ones_f = cpool.tile([1, 128], F32, tag="ones_f")
            nc.sync.dma_start(out=ones_f[:], in_=ones_e[:])
            ones = cpool.tile([1, 128], F32R, tag="ones")
            nc.vector.tensor_copy(ones[:], ones_f[:])
            bqc = cpool.tile([128, 4], F32, tag="bqc")
            nc.sync.dma_start(out=bqc[:], in_=bqc_e[:])
            bkc = cpool.tile([128, 4], F32, tag="bkc")
            nc.sync.dma_start(out=bkc[:], in_=bkc_e[:])

            iden = cpool.tile([128, 128], BF16, tag="iden")
            nc.vector.tensor_copy(iden[:], iden_f[:])
            wrvt_a = cpool.tile([128, 65], BF16, tag="wrvt_a")
            nc.vector.tensor_copy(wrvt_a[:], wrvta_f[:])
            wrvt_b = cpool.tile([4, 65], BF16, tag="wrvt_b")
            nc.vector.tensor_copy(wrvt_b[:], wrvtb_f[:])

            # ---- persistent activations ----
            qt = [ppool.tile([128, S], F32R, tag=f"qt{t}", name=f"qt{t}") for t in range(4)]
            kt = [ppool.tile([128, S + 128], F32R, tag=f"kt{t}", name=f"kt{t}") for t in range(4)]
            vsb = [ppool.tile([128, 8 * 65], BF16, tag=f"v{j}", name=f"v{j}") for j in range(ST)]

            # zero k padding columns (64 each side)
            for t in range(4):
                nc.vector.memset(kt[t][:, 0:64].bitcast(F32), 0.0)
                nc.vector.memset(kt[t][:, S + 64 : S + 128].bitcast(F32), 0.0)

            # ---- projections ----
            with (
                tc.tile_pool(name="xw", bufs=1) as xw,
                tc.tile_pool(name="proj_ps", bufs=1, space="PSUM") as pps,
            ):
                xts = [xw.tile([128, S], F32R, tag=f"x{k}", name=f"x{k}") for k in range(KT)]
                wq = [xw.tile([128, 512], F32R, tag=f"wq{k}", name=f"wq{k}") for k in range(KT)]
                wk_ = [xw.tile([128, 512], F32R, tag=f"wk{k}", name=f"wk{k}") for k in range(KT)]
                wv = [xw.tile([128, 512], F32R, tag=f"wv{k}", name=f"wv{k}") for k in range(KT)]
                for k in range(KT):
                    xs = xw.tile([128, S], F32, tag="xstage", bufs=2, name=f"xs{k}")
                    nc.sync.dma_start(out=xs[:], in_=xt_e[k * 128 : (k + 1) * 128, :])
                    nc.vector.tensor_copy(xts[k][:], xs[:])
                for k in range(KT):
                    for wdst, wsrc in ((wq[k], wqt_e), (wk_[k], wkt_e), (wv[k], wvt_e)):
                        ws = xw.tile([128, 512], F32, tag="wstage", bufs=3, name=f"ws{k}")
                        nc.sync.dma_start(out=ws[:], in_=wsrc[k * 128 : (k + 1) * 128, :])
                        nc.vector.tensor_copy(wdst[:], ws[:])

                # qT / kT feature-major [512, S]; k-outer so the PE starts as
                # soon as the first K-chunk lands, accumulating into 8 banks.
                for wsrcs, dsts, biases, seq_major in (
                    (wq, qt, bqc, False),
                    (wk_, kt, bkc, False),
                    (wv, vsb, None, True),
                ):
                    ps8 = [
                        pps.tile([128, 512], F32, tag=f"pj{i}", name=f"pj{i}")
                        for i in range(8)
                    ]
                    for k in range(KT):
                        for i in range(8):
                            if seq_major:
                                lh = xts[k][:, i * 128 : (i + 1) * 128]
                                rh = wsrcs[k][:]
                            else:
                                t, nch = i // 2, i % 2
                                lh = wsrcs[k][:, t * 128 : (t + 1) * 128]
                                rh = xts[k][:, nch * 512 : (nch + 1) * 512]
                            nc.tensor.matmul(
                                ps8[i][:], lh, rh,
                                start=(k == 0), stop=(k == KT - 1),
                            )
                    for i in range(8):
                        if seq_major:
                            dst = dsts[i][:].rearrange("p (h d) -> p h d", h=8, d=65)
                            nc.vector.tensor_copy(
                                dst[:, :, 0:64],
                                ps8[i][:].rearrange("p (h d) -> p h d", h=8, d=64),
                            )
                            nc.vector.memset(dst[:, :, 64:65], 1.0)
                        else:
                            t, nch = i // 2, i % 2
                            off = 0 if dsts is qt else 64
                            nc.scalar.activation(
                                dsts[t][:, off + nch * 512 : off + (nch + 1) * 512],
                                ps8[i][:],
                                AF.Identity,
                                bias=biases[:, t : t + 1],
                            )

            # ---- per (head) attention ----
            with (
                tc.tile_pool(name="wt", bufs=1) as wtp,
                tc.tile_pool(name="probs", bufs=2) as prp,
                tc.tile_pool(name="sm", bufs=2) as smp,
                tc.tile_pool(name="st_ps", bufs=3, space="PSUM") as stps,
                tc.tile_pool(name="ctx_ps", bufs=2, space="PSUM") as ctxps,
                tc.tile_pool(name="small_ps", bufs=1, space="PSUM") as smps,
            ):
                # Phase A (all heads up front): a_k -> batched shear-scatter.
                # GPSIMD drains these in the background; the per-head loop
                # below never waits on a scatter for its scores inserts.
                w4 = {}
                for hh in range(NHC):
                    tq = qt[hh // 2]
                    base = (hh % 2) * 64
                    for g in range(2):
                        a4 = smp.tile([128, 528], BF16, tag=f"a4_{hh}_{g}",
                                      name=f"a4_{hh}_{g}")
                        for q in range(4):
                            it = g * 4 + q
                            akps = stps.tile([128, 256], F32, tag="st")
                            nc.tensor.matmul(
                                akps[:],
                                tq[base : base + 64, it * 128 : (it + 1) * 128],
                                wrkt[base : base + 64, :],
                                start=True, stop=True,
                            )
                            nc.vector.tensor_copy(
                                a4[:, q * 132 : (q + 1) * 132], akps[:, 0:132]
                            )
                        wt4 = wtp.tile([128, 4 * 384], BF16, tag=f"w4_{hh}_{g}",
                                       name=f"w4_{hh}_{g}")
                        nc.gpsimd.local_scatter(
                            wt4[:], a4[:], insx[:],
                            channels=128, num_elems=4 * 384, num_idxs=528,
                        )
                        w4[(hh, g)] = wt4

                def wslice(hh, it, c0, c1):
                    return w4[(hh, it // 4)][:, (it % 4) * 384 + c0 : (it % 4) * 384 + c1]

                for hh in range(NHC):
                    tq = qt[hh // 2]
                    tk = kt[hh // 2]
                    base = (hh % 2) * 64

                    # B: scoresT chunks -> probsT, band pieces accumulate via
                    # bf16 transpose-matmuls (full 128-partition, zero-padded)
                    prbs = []
                    for jt in range(ST):
                        pr = prp.tile([128, S], BF16, tag=f"pr{jt}", name=f"pr{jt}")
                        for ich in range(2):
                            st = stps.tile([128, 512], F32, tag="st")
                            writes = [(
                                tk[base : base + 64, 64 + jt * 128 : 64 + (jt + 1) * 128],
                                tq[base : base + 64, ich * 512 : (ich + 1) * 512],
                                (0, 512), True,
                            )]
                            pieces = [(jt, 128)]
                            if jt > 0:
                                pieces.append((jt - 1, 256))
                            if jt < ST - 1:
                                pieces.append((jt + 1, 0))
                            for src_it, c0 in pieces:
                                if src_it // 4 == ich:
                                    lo = (src_it % 4) * 128
                                    writes.append((
                                        wslice(hh, src_it, c0, c0 + 128),
                                        iden[:], (lo, lo + 128), False,
                                    ))
                            for i, (lh, rh, (clo, chi), sflag) in enumerate(writes):
                                nc.tensor.matmul(
                                    st[:, clo:chi], lh, rh,
                                    start=sflag, stop=(i == len(writes) - 1),
                                )
                            nc.scalar.activation(
                                pr[:, ich * 512 : (ich + 1) * 512], st[:],
                                AF.Exp, bias=mbc[:, jt : jt + 1],
                            )
                        prbs.append(pr)

                    # C: PV (+ softmax sums in row 64)
                    ctx = ctxps.tile([65, S], F32, tag="ctx")
                    for jt in range(ST):
                        for ich in range(2):
                            nc.tensor.matmul(
                                ctx[:, ich * 512 : (ich + 1) * 512],
                                vsb[jt][:, hh * 65 : (hh + 1) * 65],
                                prbs[jt][:, ich * 512 : (ich + 1) * 512],
                                start=(jt == 0), stop=False,
                            )

                    # D: banded value term in [i,*] coords, two i-tiles per batch
                    for g2 in range(4):
                        bs2 = stps.tile([128, 512], F32, tag="st")
                        for h2 in range(2):
                            it = g2 * 2 + h2
                            nc.tensor.matmul(
                                bs2[:, h2 * 256 : (h2 + 1) * 256],
                                tq[base : base + 64, it * 128 : (it + 1) * 128],
                                tk[base : base + 64, it * 128 : it * 128 + 256],
                                start=True, stop=not with_mask_bias,
                            )
                            if with_mask_bias:
                                nc.tensor.matmul(
                                    bs2[:, h2 * 256 : (h2 + 1) * 256],
                                    ones[0:1, :],
                                    mbp[0:1, it * 128 : it * 128 + 256],
                                    start=False, stop=True,
                                )
                            nc.vector.tensor_add(
                                bs2[:, h2 * 256 : (h2 + 1) * 256],
                                bs2[:, h2 * 256 : (h2 + 1) * 256],
                                wslice(hh, it, 64, 320),
                            )
                        ppu = smp.tile([128, 512], BF16, tag="ppu")
                        nc.scalar.activation(ppu[:], bs2[:], AF.Exp)
                        if g2 == 0:
                            nc.vector.memset(ppu[:, 0:64], 0.0)
                        if g2 == 3:
                            nc.vector.memset(ppu[:, 448:512], 0.0)
                        pp = smp.tile([128, 512], BF16, tag="pp")
                        nc.gpsimd.local_scatter(
                            pp[:], ppu[:], valx[:],
                            channels=128, num_elems=512, num_idxs=512,
                        )
                        for h2 in range(2):
                            it = g2 * 2 + h2
                            pa = smp.tile([128, 128], BF16, tag="pa")
                            nc.sync.dma_start_transpose(
                                pa[:], pp[:, h2 * 256 : h2 * 256 + 128]
                            )
                            ppt = smps.tile([4, 128], F32, tag="sm")
                            nc.tensor.matmul(
                                ppt[:], pp[:, h2 * 256 + 128 : h2 * 256 + 132],
                                iden[:], start=True, stop=True,
                            )
                            pb = smp.tile([4, 128], BF16, tag="pb")
                            nc.vector.tensor_copy(pb[:], ppt[:])
                            nc.tensor.matmul(
                                ctx[0:65, it * 128 : (it + 1) * 128],
                                wrvt_a[:], pa[:], start=False, stop=False,
                            )
                            nc.tensor.matmul(
                                ctx[0:65, it * 128 : (it + 1) * 128],
                                wrvt_b[:], pb[:], start=False,
                                stop=(it == 3 or it == ST - 1),
                            )

                    # E: copy ctx out of PSUM (frees it for the next head),
                    # then normalize entirely off the critical path.
                    cs = smp.tile([65, S], F32, tag="cs")
                    nc.vector.tensor_copy(cs[:], ctx[:])
                    r = smp.tile([1, S], F32R, tag="r")
                    nc.vector.reciprocal(r[:], cs[64:65, :])
                    rb = ctxps.tile([64, S], F32, tag="ctx", name="rb")
                    for ich in range(2):
                        nc.tensor.matmul(
                            rb[:, ich * 512 : (ich + 1) * 512],
                            ones[0:1, 0:64],
                            r[0:1, ich * 512 : (ich + 1) * 512],
                            start=True, stop=True,
                        )
                    o = smp.tile([64, S], F32, tag="o")
                    nc.vector.tensor_mul(o[:], cs[0:64, :], rb[:])
                    nc.sync.dma_start(out=out_e[hh], in_=o[:])

    nc.compile()
    return nc


_NC_CACHE = {}


def _get_nc(with_mask_bias=False):
    if with_mask_bias not in _NC_CACHE:
        _NC_CACHE[with_mask_bias] = _build_nc(with_mask_bias)
    return _NC_CACHE[with_mask_bias]


def _host_prep(inputs):
    hs = np.asarray(inputs["hidden_states"], np.float32)
    am = np.asarray(inputs["attention_mask"], np.float32)
    Wq = np.asarray(inputs["Wq"], np.float32)
    bq = np.asarray(inputs["bq"], np.float32)
    Wk = np.asarray(inputs["Wk"], np.float32)
    bk = np.asarray(inputs["bk"], np.float32)
    Wv = np.asarray(inputs["Wv"], np.float32)
    Wrk = np.asarray(inputs["Wrk"], np.float32)
    Wrv = np.asarray(inputs["Wrv"], np.float32)

    wrkt = np.zeros((128, 256), np.float32)
    wrkt[0:64, 0:NW] = Wrk.T
    wrkt[64:128, 0:NW] = Wrk.T
    wrvt = np.zeros((132, 65), np.float32)
    wrvt[0:NW, 0:64] = Wrv.T
    wrvt_a = np.ascontiguousarray(wrvt[0:128])
    wrvt_b = np.ascontiguousarray(wrvt[128:132])

    iden = np.eye(128, dtype=np.float32)
    p = np.arange(128)[:, None]
    j = np.arange(528)[None, :]
    q, w = j // 132, j % 132
    ins_idx = np.where(w <= 128, q * 384 + p + w + 64, -1).astype(np.int16)
    c = np.arange(512)[None, :]
    h, cc = c // 256, c % 256
    dd = cc - p
    val_idx = np.where((dd >= 0) & (dd <= 128), h * 256 + dd, -1).astype(np.int16)
    ones_row = np.ones((1, 128), np.float32)

    in_maps = []
    for core in range(NCORES):
        b = core // 2
        h0 = (core % 2) * NHC
        fsl = slice(h0 * DH, h0 * DH + 512)
        mb = (1.0 - am[b]) * NEG
        mbp = np.zeros((1, S + 128), np.float32)
        mbp[0, 64 : 64 + S] = mb
        in_maps.append({
            "xt": np.ascontiguousarray(hs[b].T),
            "wqt": np.ascontiguousarray(Wq[fsl].T),
            "wkt": np.ascontiguousarray(Wk[fsl].T),
            "wvt": np.ascontiguousarray(Wv[fsl].T),
            "bq_cols": np.ascontiguousarray(bq[fsl].reshape(4, 128).T),
            "bk_cols": np.ascontiguousarray(bk[fsl].reshape(4, 128).T),
            "wrkt": wrkt,
            "wrvt_a": wrvt_a,
            "wrvt_b": wrvt_b,
            "mbias_pad": mbp,
            "mbias_cols": np.ascontiguousarray(mb.reshape(ST, 128).T),
            "identity": iden,
            "ins_idx": ins_idx,
            "val_idx": val_idx,
            "ones_row": ones_row,
        })
    return in_maps


def _assemble(results, inputs):
    bv = np.asarray(inputs["bv"], np.float32)
    full = np.empty((B, S, H * DH), np.float32)
    for core in range(NCORES):
        b = core // 2
        h0 = (core % 2) * NHC
        o = results[core]["out"]  # [NHC, DH, S]
        for hh in range(NHC):
            h = h0 + hh
            full[b, :, h * DH : (h + 1) * DH] = o[hh].T
    full += bv[None, None, :]
    return full


def kernel(**inputs):
    global LAST_EXEC_NS, LAST_RESULTS
    mask_all_ones = bool(np.all(np.asarray(inputs["attention_mask"]) == 1.0))
    nc = _get_nc(with_mask_bias=not mask_all_ones)
    in_maps = _host_prep(inputs)
    trace = bool(int(os.environ.get("KERNEL_TRACE", "0")))
    res = bass_utils.run_bass_kernel_spmd(
        nc, in_maps, core_ids=list(range(NCORES)), trace=trace
    )
    LAST_EXEC_NS = res.exec_time_ns
    LAST_RESULTS = res
    return _assemble(res.results, inputs)
